# revision 62
# baseline (speedup 1.0000x reference)
# Trainium2 Bass kernel for nn_FPN_AAR (dense_cnn): 5-branch deformable-sampled
# 3x3 conv + SKConv attention fusion, sharded over 8 NeuronCores by output rows.
#
# v2: fp8(e4m3) DoubleRow convs at 0.5 cyc/row (2x the fp32r rate) with
# error-compensated decompositions (psum scale 1024 = 64*16):
#   BN branches (45/90/135/180):  1024*y = W_h @ x_h + W_l @ x_h
#   raw-added branch (angle 0):   1024*y = W_h @ x_h + W_h @ x_l + W_l @ x_h
# where W_h = q8(64 W), W_l = q8(64 (W - W_h/64)), x_h = q8(16 xo),
# x_l = q8(16 xo - x_h). Each DoubleRow matmul contracts both 128-channel
# halves (lhsT [128,2,128], rhs [128,2,432] flat-row windows, 2 junk columns
# per output row). The BN branches' quantization error is attenuated ~3x by
# softmax attention (~0.25/branch) + ReLU; angle 0 is added raw so it gets
# the full 3-term treatment. Measured hw rel err 1.27e-2 (gate 2e-2),
# bit-stable across runs and exactly matching numpy emulation.
#
#   - Each core computes an 18-row slab of the 142-row output for all 5
#     branches (core 7 overhangs; host keeps its valid 16 rows).
#   - Sampling on-chip: integer angles gather host-prequantized fp8 tensors;
#     fractional angles build a x16-scaled f32 bilerp then quantize+residual.
#     Angle order 90,45,180,135 gives the second fractional sampling two
#     conv spans of aux-engine time.
#   - Branch outputs (feas) stay in SBUF as fp16; BN+ReLU on ScalarE with
#     accum_out providing the spatial sums for the attention path.
#   - Cross-core: masked AllReduce of the pooled sums (core 7's overhang
#     rows zeroed via a per-core mask input); every core redundantly runs
#     the tiny fc1/fc2/softmax.
#   - Endgame: attention weights carry the 1024 psum scale, the per-group
#     attention-weighted branch sum s accumulates in-place into feas[m=0],
#     y0 banks stage to fp16 (or add s directly from PSUM in the last
#     phase), and the host divides the DMA'd output by 1024.
#   - PSUM is only ever read by ScalarE/DVE (GPSIMD cannot access PSUM).
import sys
import math

sys.path.insert(0, "/opt/trn_rl_repo")

import numpy as np
import ml_dtypes

F8 = ml_dtypes.float8_e4m3

KS = 3
S2 = 2**0.5
ANGLES = (0, 45, 90, 135, 180)
BN_EPS = 1e-5
NCORES = 8
B, C, H, W = 2, 256, 48, 48
HOUT = 142
NROWS = 18      # output rows per core slab
NG = 6          # row groups of 3
I0S = [6 * k for k in range(7)] + [42]
SX = 16.0       # x quant scale
SW = 64.0       # weight quant scale
SOUT = SX * SW  # psum scale

# per-angle conv term plan: "hh" = W_h@x_h, "hl" = W_h@x_l (x-residual
# correction), "lh" = W_l@x_h (weight-residual correction)
TERM_PLAN = {a: (("hh", "hl", "lh") if a == 0 else ("hh", "lh"))
             for a in ANGLES}


def _angle_offsets(angle):
    n = angle // 45
    if n == 0:
        ox = [0.0] * 9
        oy = [0.0] * 9
    elif n == 1:
        ox = [1 - S2, 1 - S2 * 0.5, 1, -S2 * 0.5, 0, S2 * 0.5, -1, S2 * 0.5 - 1, S2 - 1]
        oy = [1, S2 * 0.5, S2 - 1, 1 - S2 * 0.5, 0, S2 * 0.5 - 1, 1 - S2, -S2 * 0.5, -1]
    elif n == 2:
        ox = [0, 1, 2, -1, 0, 1, -2, -1, 0]
        oy = [2, 1, 0, 1, 0, -1, 0, -1, -2]
    elif n == 3:
        ox = [1, 1 + S2 * 0.5, 1 + S2, -S2 * 0.5, 0, S2 * 0.5, -1 - S2, -1 - S2 * 0.5, -1]
        oy = [1 + S2, S2 * 0.5, -1, 1 + S2 * 0.5, 0, -1 - S2 * 0.5, 1, -S2 * 0.5, 1 + S2]
    else:
        ox = [2, 2, 2, 0, 0, 0, -2, -2, -2]
        oy = [2, 0, -2, 2, 0, -2, 2, 0, -2]
    return ox, oy


def _angle_terms(angle):
    """Per kernel point p=(r,s): sampling as up to 2x2 separable terms.

    xo[c, 3I+r, 3j+s] = sum_u sum_v wx_u*wy_v * xs[c, I+dx_u+4, j+dy_v+4]
    where the input slab xs carries zero margins so clipped samples read 0,
    matching the reference's clamp-into-padding semantics exactly.
    """
    ox, oy = _angle_offsets(angle)
    pn = [-1.0, 0.0, 1.0]
    out = {}
    for p in range(9):
        cx = 1.0 + pn[p // 3] + float(ox[p])
        cy = 1.0 + pn[p % 3] + float(oy[p])
        fx, fy = math.floor(cx), math.floor(cy)
        ax, ay = cx - fx, cy - fy
        xterms = [(int(d), w) for d, w in [(fx, 1 - ax), (fx + 1, ax)] if w != 0.0]
        yterms = [(int(d), w) for d, w in [(fy, 1 - ay), (fy + 1, ay)] if w != 0.0]
        out[p] = (xterms, yterms)
    return out


_BUILD_CACHE = {}


def _build_program():
    key = ("nc", repr(sorted(TERM_PLAN.items())))
    if key in _BUILD_CACHE:
        return _BUILD_CACHE[key]

    import concourse.bacc as bacc
    import concourse.tile as tile
    import concourse.mybir as mybir

    f32 = mybir.dt.float32
    f16 = mybir.dt.float16
    f8 = mybir.dt.float8e4
    AF = mybir.ActivationFunctionType
    ALU = mybir.AluOpType
    AX = mybir.AxisListType
    DR = mybir.MatmulPerfMode.DoubleRow

    nc = bacc.Bacc("TRN2", target_bir_lowering=False, debug=False,
                   num_devices=NCORES)

    xs_d = nc.dram_tensor("xs", [128, 4, 16, 57], f32, kind="ExternalInput")
    x8h_d = nc.dram_tensor("x8h", [128, 4, 16, 57], f8, kind="ExternalInput")
    x8l_d = nc.dram_tensor("x8l", [128, 4, 16, 57], f8, kind="ExternalInput")
    w8h_d = nc.dram_tensor("w8h", [128, 2, 9, 2, 128], f8, kind="ExternalInput")
    w8l_d = nc.dram_tensor("w8l", [128, 2, 9, 2, 128], f8, kind="ExternalInput")
    bias_d = nc.dram_tensor("bias_act", [128, 2, 4], f32, kind="ExternalInput")
    gpr_d = nc.dram_tensor("gpr", [128, 2, 4], f32, kind="ExternalInput")
    w1t_d = nc.dram_tensor("w1t", [128, 2, 32], f32, kind="ExternalInput")
    b1_d = nc.dram_tensor("b1", [32, 1], f32, kind="ExternalInput")
    w2t_d = nc.dram_tensor("w2t", [32, 4, 2, 128], f32, kind="ExternalInput")
    b2t_d = nc.dram_tensor("b2t", [128, 2, 4], f32, kind="ExternalInput")
    m7_d = nc.dram_tensor("m7", [128, 1], f32, kind="ExternalInput")
    out_d = nc.dram_tensor("out", [128, 4, NROWS, HOUT], f32, kind="ExternalOutput")

    terms = {a: _angle_terms(a) for a in ANGLES}
    branch_of = {45: 0, 90: 1, 135: 2, 180: 3}

    with tile.TileContext(nc) as tc:
        with tc.tile_pool(name="persist", bufs=1) as pp, \
             tc.tile_pool(name="xof", bufs=1) as xofp, \
             tc.tile_pool(name="xor", bufs=3) as xorp, \
             tc.tile_pool(name="work", bufs=2) as wp, \
             tc.tile_pool(name="apply", bufs=2) as app, \
             tc.tile_pool(name="psum", bufs=8, space="PSUM") as psp, \
             tc.tile_pool(name="dram", bufs=1, space="DRAM") as dp:

            # ---- persistent loads ----
            # first-conv critical path: x8h gathers + w8h weights
            x8h_t = pp.tile([128, 4, 16, 57], f8, tag="x8h")
            nc.sync.dma_start(x8h_t[:], x8h_d.ap()[:])
            w8h_t = pp.tile([128, 2, 9, 2, 128], f8, tag="w8h")
            nc.sync.dma_start(w8h_t[:], w8h_d.ap()[:])
            w8l_t = pp.tile([128, 2, 9, 2, 128], f8, tag="w8l")
            nc.sync.dma_start(w8l_t[:], w8l_d.ap()[:])
            x8l_t = pp.tile([128, 4, 16, 57], f8, tag="x8l")
            nc.sync.dma_start(x8l_t[:], x8l_d.ap()[:])
            xs_t4 = []
            for _bc in range(4):
                xst = pp.tile([128, 16, 57], f32, tag=f"xs{_bc}",
                              name=f"xs{_bc}")
                nc.sync.dma_start(xst[:], xs_d.ap()[:, _bc])
                xs_t4.append(xst)
            xs_t = [t[:] for t in xs_t4]
            bias_sb = pp.tile([128, 2, 4], f32, tag="bias")
            nc.sync.dma_start(bias_sb[:], bias_d.ap()[:])
            gpr_sb = pp.tile([128, 2, 4], f32, tag="gpr")
            nc.sync.dma_start(gpr_sb[:], gpr_d.ap()[:])
            w1t_sb = pp.tile([128, 2, 32], f32, tag="w1t")
            nc.sync.dma_start(w1t_sb[:], w1t_d.ap()[:])
            b1_sb = pp.tile([32, 1], f32, tag="b1")
            nc.sync.dma_start(b1_sb[:], b1_d.ap()[:])
            w2t_sb = pp.tile([32, 4, 2, 128], f32, tag="w2t")
            nc.sync.dma_start(w2t_sb[:], w2t_d.ap()[:])
            b2t_sb = pp.tile([128, 2, 4], f32, tag="b2t")
            nc.sync.dma_start(b2t_sb[:], b2t_d.ap()[:])
            m7_sb = pp.tile([128, 1], f32, tag="m7")
            nc.sync.dma_start(m7_sb[:], m7_d.ap()[:])

            # PE warm-up: dependency-free matmuls on zeroed tiles run while
            # the input DMAs land, releasing the HAM clock gate before the
            # first real conv matmul.
            wz = pp.tile([128, 512], mybir.dt.bfloat16, tag="warmz")
            nc.vector.memset(wz[:], 0.0)
            wps = psp.tile([128, 512], f32, tag="ps", name="warm_ps")
            for _wi in range(16):
                nc.tensor.matmul(wps[:], wz[:, 0:128], wz[:],
                                 start=True, stop=True)

            # acc layout: [m(4), b(2), oc(2), acc(8)]
            # acc 0..4 = row-group sums g0..g4, acc 5..7 = rows 15,16,17
            acc_sb = pp.tile([128, 4, 2, 2, 8], f32, tag="acc")

            # feas/y0 resident in SBUF as fp16
            feas_t = {}
            for m in range(4):
                for bb in range(2):
                    for oc in range(2):
                        t = pp.tile([128, NG, 3, HOUT], f16,
                                    tag=f"feas_{m}_{bb}_{oc}",
                                    name=f"feas_{m}_{bb}_{oc}")
                        feas_t[(m, bb, oc)] = t

            y0tmp_t = {}
            for bb in range(2):
                for oc in range(2):
                    t = pp.tile([128, NG, 3, HOUT], f16,
                                tag=f"y0tmp_{bb}_{oc}",
                                name=f"y0tmp_{bb}_{oc}")
                    y0tmp_t[(bb, oc)] = t

            ag_in = dp.tile([128, 128], f32, tag="ag_in")
            ar_out = dp.tile([128, 128], f32, tag="ar_out")

            attp = pp.tile([128, 2, 2, 4], f32, tag="attp")

            # ---- sampling ----
            def emit_sample(a):
                """Build xr_h/xr_l fp8 tiles [128, 2(cc), 21, 144] per bb."""
                integer_angle = all(
                    len(terms[a][p][0]) == 1 and len(terms[a][p][1]) == 1
                    for p in range(9))
                # Act carries only bank-paced drains; sampling lives on
                # DVE+Pool so it never queues behind a previous angle's
                # drain retirement
                copy_engs = [
                    lambda d, s: nc.gpsimd.tensor_copy(d, s),
                    lambda d, s: nc.vector.tensor_copy(d, s),
                ]
                if a == 90:
                    # first angle: Act has no pending drains yet
                    copy_engs.append(lambda d, s: nc.scalar.copy(d, s))
                need_l = "hl" in TERM_PLAN[a]
                kinds = (("h", x8h_t), ("l", x8l_t)) if need_l \
                    else (("h", x8h_t),)
                xr = {"h": [], "l": []}
                if integer_angle:
                    ei = 0
                    for bb in range(2):
                        for kind, src8 in kinds:
                            t = xorp.tile([128, 2, 21, 144], f8,
                                          tag=f"xr{kind}", bufs=4
                                          if kind == "h" else 2,
                                          name=f"xr_{a}_{kind}_{bb}")
                            xr[kind].append(t)
                            for cc in range(2):
                                s8 = src8[:, bb * 2 + cc]
                                for p in range(9):
                                    r, s = p // 3, p % 3
                                    dx = terms[a][p][0][0][0]
                                    dy = terms[a][p][1][0][0]
                                    copy_engs[ei % len(copy_engs)](
                                        t[:, cc, r::3, s::3],
                                        s8[:, 4 + dx:4 + dx + 7,
                                           4 + dy:4 + dy + 48])
                                    ei += 1
                    return xr
                # fractional angle: f32 bilerp at x16 scale, then quantize
                for bb in range(2):
                    th = xorp.tile([128, 2, 21, 144], f8, tag="xrh",
                                   bufs=4, name=f"xr_{a}_h_{bb}")
                    xr["h"].append(th)
                    tl = None
                    if need_l:
                        tl = xorp.tile([128, 2, 21, 144], f8, tag="xrl",
                                       bufs=2, name=f"xr_{a}_l_{bb}")
                        xr["l"].append(tl)
                    for cc in range(2):
                        xsl = xs_t[bb * 2 + cc]          # [128, 16, 57]
                        xof = xofp.tile([128, 21, 144], f32, tag="xof",
                                        name=f"xof_{a}_{bb}_{cc}")
                        acache = {}
                        for p in range(9):
                            r, s = p // 3, p % 3
                            xterms, yterms = terms[a][p]
                            dst = xof[:, r::3, s::3]        # [128, 7, 48]
                            if len(xterms) == 1:
                                dx, wx = xterms[0]
                                At = xsl[:, 4 + dx:4 + dx + 7, :]
                                ascale = wx
                            else:
                                key = tuple(xterms)
                                if key in acache:
                                    At = acache[key][:]
                                else:
                                    (dx0, wx0), (dx1, wx1) = xterms
                                    Atile = wp.tile([128, 7, 57], f32,
                                                    tag="stepA", bufs=2,
                                                    name=f"sa_{a}_{bb}_{cc}_{p}")
                                    t2 = wp.tile([128, 7, 57], f32,
                                                 tag="stepA2", bufs=1,
                                                 name=f"sa2_{a}_{bb}_{cc}_{p}")
                                    nc.vector.tensor_scalar_mul(
                                        Atile[:], xsl[:, 4 + dx0:4 + dx0 + 7, :],
                                        float(wx0))
                                    nc.gpsimd.tensor_scalar_mul(
                                        t2[:], xsl[:, 4 + dx1:4 + dx1 + 7, :],
                                        float(wx1))
                                    nc.vector.tensor_add(Atile[:], Atile[:], t2[:])
                                    acache[key] = Atile
                                    At = Atile[:]
                                ascale = 1.0
                            if len(yterms) == 1:
                                dy, wy = yterms[0]
                                w = SX * ascale * wy
                                nc.gpsimd.tensor_scalar_mul(
                                    dst, At[:, :, 4 + dy:4 + dy + 48],
                                    float(w))
                            else:
                                (dy0, wy0), (dy1, wy1) = yterms
                                tb = wp.tile([128, 7, 48], f32, tag="stepB",
                                             bufs=1, name=f"sb_{a}_{bb}_{cc}_{p}")
                                nc.vector.tensor_scalar_mul(
                                    dst, At[:, :, 4 + dy0:4 + dy0 + 48],
                                    float(SX * ascale * wy0))
                                nc.vector.tensor_scalar_mul(
                                    tb[:], At[:, :, 4 + dy1:4 + dy1 + 48],
                                    float(SX * ascale * wy1))
                                nc.vector.tensor_add(dst, dst, tb[:])
                        # quantize: h = q8(xof16); l = q8(xof16 - h)
                        # split each over engines by row range for balance
                        nc.vector.tensor_copy(th[:, cc, 0:11, :],
                                              xof[:, 0:11, :])
                        nc.gpsimd.tensor_copy(th[:, cc, 11:21, :],
                                              xof[:, 11:21, :])
                        if need_l:
                            nc.vector.tensor_tensor(
                                tl[:, cc, 0:11, :], xof[:, 0:11, :],
                                th[:, cc, 0:11, :], ALU.subtract)
                            nc.gpsimd.tensor_tensor(
                                tl[:, cc, 11:21, :], xof[:, 11:21, :],
                                th[:, cc, 11:21, :], ALU.subtract)
                return xr

            # ---- conv ----
            def emit_conv(a, bb, oc, xr, drain=None, inject=None):
                banks = [psp.tile([128, 3, 144], f32, tag="ps",
                                  name=f"ps_{a}_{bb}_{oc}_{g}")
                         for g in range(NG)]
                outs = [b_[:].rearrange("p a b -> p (a b)") for b_ in banks]
                rh = xr["h"][bb][:].rearrange("p c h w -> p c (h w)")
                wh = w8h_t[:, oc]                    # [128, 9, 2, 128]
                wl = w8l_t[:, oc]
                terms_of = {"hh": (wh, rh), "lh": (wl, rh)}
                if "hl" in TERM_PLAN[a]:
                    rl = xr["l"][bb][:].rearrange("p c h w -> p c (h w)")
                    terms_of["hl"] = (wh, rl)
                plan = [terms_of[t] for t in TERM_PLAN[a]]
                # g-outer: finish one PSUM bank at a time so drains (and the
                # next phase's bank allocations) chase the matmul stream
                # instead of serializing at phase end.
                for g in range(NG):
                    for tap in range(9):
                        di, dj = tap // 3, tap % 3
                        off = 432 * g + 144 * di + dj
                        for ti, (wt, rr) in enumerate(plan):
                            lhsT = wt[:, tap]        # [128, 2, 128]
                            first = (tap == 0 and ti == 0)
                            last = (tap == 8 and ti == len(plan) - 1)
                            nc.tensor.matmul(outs[g], lhsT,
                                             rr[:, :, off:off + 432],
                                             start=first, stop=last,
                                             perf_mode=DR)
                    if drain is not None:
                        drain(g, banks[g])
                    if inject is not None and g in inject:
                        inject[g]()
                return banks

            def drain_branch(a, bb, oc):
                m = branch_of[a]
                ft = feas_t[(m, bb, oc)]

                def drain(g, bank):
                    if g < 5:
                        nc.scalar.activation(
                            ft[:, g], bank[:, :, 0:HOUT], AF.Relu,
                            bias=bias_sb[:, oc, m:m + 1],
                            scale=gpr_sb[:, oc, m:m + 1],
                            accum_out=acc_sb[:, m, bb, oc, g:g + 1])
                    else:
                        for r in range(3):
                            nc.scalar.activation(
                                ft[:, 5, r], bank[:, r, 0:HOUT], AF.Relu,
                                bias=bias_sb[:, oc, m:m + 1],
                                scale=gpr_sb[:, oc, m:m + 1],
                                accum_out=acc_sb[:, m, bb, oc, 5 + r:6 + r])
                return drain

            def copy_drain(bb, oc, also=(), off_act=False):
                # stage raw PSUM (scale SOUT, fits fp16) so banks free
                # immediately; the host divides the output by SOUT
                yt = y0tmp_t[(bb, oc)]

                def drain(g, bank):
                    if off_act:
                        # PSUM reads: DVE only (GPSIMD cannot access PSUM)
                        nc.vector.tensor_copy(yt[:, g], bank[:, :, 0:HOUT])
                    else:
                        nc.scalar.copy(yt[:, g], bank[:, :, 0:HOUT])
                    for f in also:
                        f(g)
                return drain

            def emit_collective_reduce():
                # mask the rows-16/17 accum slots on core 7 (overhang rows),
                # then AllReduce so the cross-core sum arrives ready-made
                nc.vector.tensor_scalar_mul(
                    acc_sb[:, :, :, :, 6:8],
                    acc_sb[:, :, :, :, 6:8], m7_sb[:, 0:1])
                nc.sync.dma_start(
                    ag_in[:, :],
                    acc_sb[:].rearrange("p a b c d -> p (a b c d)"))
                nc.gpsimd.collective_compute(
                    "AllReduce", ALU.add,
                    replica_groups=[list(range(NCORES))],
                    ins=[ag_in.opt()], outs=[ar_out.opt()])
                ar_sb = pp.tile([128, 16, 8], f32, tag="ar_sb")
                nc.sync.dma_start(
                    ar_sb[:],
                    ar_out[:].rearrange("p (mbo a) -> p mbo a", a=8))
                fm = pp.tile([128, 16], f32, tag="fm")
                nc.vector.reduce_sum(fm[:], ar_sb[:], axis=AX.X)
                # fm layout [m, b, oc]; feas sums are already BN'd
                fmv = fm[:].rearrange("p (m b o) -> p m b o", m=4, b=2)
                fs = pp.tile([128, 2, 2], f32, tag="fs")
                nc.vector.reduce_sum(fs[:], fmv.transpose([0, 2, 3, 1]),
                                     axis=AX.X)
                nc.vector.tensor_scalar_mul(fs[:], fs[:], 1.0 / (HOUT * HOUT))
                return fs

            def emit_fc_softmax(fs):
                pz = psp.tile([32, 2], f32, tag="ps", name="pz_fc1")
                for cc in range(2):
                    nc.tensor.matmul(pz[:], w1t_sb[:, cc, :], fs[:, :, cc],
                                     start=(cc == 0), stop=(cc == 1))
                zt = pp.tile([32, 2], f32, tag="zt")
                nc.vector.tensor_scalar(zt[:], pz[:], b1_sb[:, 0:1], None,
                                        ALU.add)
                logit = pp.tile([128, 2, 2, 4], f32, tag="logit")
                for m in range(4):
                    for oc in range(2):
                        p2 = psp.tile([128, 2], f32, tag="ps",
                                      name=f"p2_fc2_{m}_{oc}")
                        nc.tensor.matmul(p2[:], w2t_sb[:, m, oc, :], zt[:],
                                         start=True, stop=True)
                        nc.vector.tensor_scalar(logit[:, oc, :, m], p2[:],
                                                b2t_sb[:, oc, m:m + 1], None,
                                                ALU.add)
                rmax = pp.tile([128, 2, 2, 1], f32, tag="rmax")
                nc.vector.tensor_reduce(rmax[:], logit[:], AX.X, ALU.max)
                nc.gpsimd.tensor_tensor(
                    logit[:], logit[:],
                    rmax[:].broadcast_to([128, 2, 2, 4]), ALU.subtract)
                elog = pp.tile([128, 2, 2, 4], f32, tag="elog")
                nc.scalar.activation(elog[:], logit[:], AF.Exp)
                ssum = pp.tile([128, 2, 2, 1], f32, tag="ssum")
                nc.vector.reduce_sum(ssum[:], elog[:], axis=AX.X)
                rinv = pp.tile([128, 2, 2, 1], f32, tag="rinv")
                nc.vector.reciprocal(rinv[:], ssum[:])
                # fold the PSUM scale into att: s = SOUT * sum(att*feas), so
                # the final output is (bank + s) and the host divides by SOUT
                nc.vector.tensor_scalar_mul(rinv[:], rinv[:], float(SOUT))
                nc.gpsimd.tensor_tensor(
                    attp[:], elog[:],
                    rinv[:].broadcast_to([128, 2, 2, 4]), ALU.mult)

            def s_group(bb, oc, variant):
                """Per-group att-weighted branch sum into feas_t[(0,bb,oc)].
                Engine budget per hosting bank: Act 2+1, DVE 4+4, Pool 1+2."""
                att = attp[:, oc, bb]
                f0 = feas_t[(0, bb, oc)]

                def emit(g):
                    t1 = app.tile([128, 3, HOUT], f16, tag="t1_t", bufs=2,
                                  name=f"t1_{bb}_{oc}_{g}")
                    t2 = app.tile([128, 3, HOUT], f16, tag="t2_t", bufs=2,
                                  name=f"t2_{bb}_{oc}_{g}")
                    t3 = app.tile([128, 3, HOUT], f16, tag="t3_t", bufs=2,
                                  name=f"t3_{bb}_{oc}_{g}")
                    if variant == 0:
                        nc.vector.tensor_scalar_mul(f0[:, g], f0[:, g],
                                                    att[:, 0:1])
                        nc.scalar.mul(t1[:], feas_t[(1, bb, oc)][:, g],
                                      att[:, 1:2])
                        nc.scalar.mul(t2[:], feas_t[(2, bb, oc)][:, g],
                                      att[:, 2:3])
                        nc.scalar.mul(t3[:], feas_t[(3, bb, oc)][:, g],
                                      att[:, 3:4])
                        nc.vector.tensor_add(f0[:, g], f0[:, g], t1[:])
                        nc.gpsimd.tensor_add(t2[:], t2[:], t3[:])
                        nc.vector.tensor_add(f0[:, g], f0[:, g], t2[:])
                    else:
                        nc.vector.tensor_scalar_mul(f0[:, g], f0[:, g],
                                                    att[:, 0:1])
                        nc.scalar.mul(t1[:], feas_t[(1, bb, oc)][:, g],
                                      att[:, 1:2])
                        nc.scalar.mul(t2[:], feas_t[(2, bb, oc)][:, g],
                                      att[:, 2:3])
                        nc.vector.tensor_scalar_mul(
                            t3[:], feas_t[(3, bb, oc)][:, g], att[:, 3:4])
                        nc.vector.tensor_add(f0[:, g], f0[:, g], t1[:])
                        nc.vector.tensor_add(t2[:], t2[:], t3[:])
                        nc.vector.tensor_add(f0[:, g], f0[:, g], t2[:])
                return emit

            def outf_drain(bb, oc):
                # last phase only: out = bank + s directly from PSUM
                f0 = feas_t[(0, bb, oc)]

                state = {}

                def drain(g, bank):
                    p = g // 2
                    if g % 2 == 0:
                        of2 = app.tile([128, 2, 3, HOUT], f32, tag="outf2",
                                       bufs=2, name=f"of1p_{bb}_{oc}_{p}")
                        state[p] = of2
                    else:
                        of2 = state.pop(p)
                    nc.vector.tensor_tensor(of2[:, g % 2], bank[:, :, 0:HOUT],
                                            f0[:, g], ALU.add)
                    if g % 2 == 1:
                        nc.sync.dma_start(
                            out_d.ap()[:, bb * 2 + oc, 6 * p:6 * p + 6, :],
                            of2[:])
                return drain

            def emit_outf_pair(bb, oc, p, eng):
                of2 = app.tile([128, 2, 3, HOUT], f32, tag="outf2", bufs=2,
                               name=f"of2_{bb}_{oc}_{p}")
                eng.tensor_tensor(of2[:], y0tmp_t[(bb, oc)][:, 2*p:2*p+2],
                                  feas_t[(0, bb, oc)][:, 2*p:2*p+2],
                                  ALU.add)
                nc.sync.dma_start(
                    out_d.ap()[:, bb * 2 + oc, 6 * p:6 * p + 6, :], of2[:])

            # ---- main schedule ----
            def emit_branch_convs(a, xr):
                for bb in range(2):
                    for oc in range(2):
                        emit_conv(a, bb, oc, xr,
                                  drain=drain_branch(a, bb, oc))

            xr90 = emit_sample(90)
            xr45 = emit_sample(45)
            emit_branch_convs(90, xr90)
            emit_branch_convs(45, xr45)
            xr180 = emit_sample(180)
            xr135 = emit_sample(135)
            emit_branch_convs(180, xr180)
            emit_branch_convs(135, xr135)

            xr0 = emit_sample(0)
            fs = emit_collective_reduce()

            emit_conv(0, 0, 0, xr0, drain=copy_drain(0, 0),
                      inject={3: lambda: emit_fc_softmax(fs)})
            for g in range(NG):
                s_group(0, 0, g % 2)(g)
                s_group(0, 1, 1 - g % 2)(g)
            emit_conv(0, 0, 1, xr0, drain=copy_drain(0, 1))
            for g in range(NG):
                s_group(1, 0, g % 2)(g)
                s_group(1, 1, 1 - g % 2)(g)
            for p in range(3):
                emit_outf_pair(0, 0, p, nc.vector if p % 2 else nc.gpsimd)
            for p in range(3):
                emit_outf_pair(0, 1, p, nc.gpsimd if p % 2 else nc.vector)
            emit_conv(0, 1, 0, xr0, drain=copy_drain(1, 0, off_act=True))
            for p in range(3):
                emit_outf_pair(1, 0, p, nc.vector if p % 2 else nc.gpsimd)
            emit_conv(0, 1, 1, xr0, drain=outf_drain(1, 1))

    nc.compile()
    _BUILD_CACHE[key] = nc
    return nc


def _q8(a):
    return np.asarray(a, np.float32).astype(F8)


def _host_prep(x, conv_w, bn_gamma, bn_beta, bn_mean, bn_var, fc1_w, fc1_b,
               fc2_w, fc2_b):
    x = np.asarray(x, np.float32)
    conv_w = np.asarray(conv_w, np.float32)
    x_ext = np.zeros((B, C, 60, 57), np.float32)
    x_ext[:, :, 5:53, 5:53] = x
    x16 = SX * x_ext
    x8h = _q8(x16)
    x8l = _q8(x16 - x8h.astype(np.float32))

    gprime = (np.asarray(bn_gamma) / np.sqrt(np.asarray(bn_var) + BN_EPS)) \
        .astype(np.float32)
    bprime = (np.asarray(bn_beta) - np.asarray(bn_mean) * gprime) \
        .astype(np.float32)

    # wt[ci, oc, tap, cc, co] = conv_w[oc*128+co, cc*128+ci, tap]
    w9 = conv_w.reshape(2, 128, 2, 128, 9)           # [oc, co, cc, ci, tap]
    wt = np.ascontiguousarray(w9.transpose(3, 0, 4, 2, 1))  # [ci,oc,tap,cc,co]
    w8h = _q8(SW * wt)
    w8l = _q8(SW * (wt - w8h.astype(np.float32) / SW))

    bias_t = np.ascontiguousarray(
        bprime.reshape(4, 2, 128).transpose(2, 1, 0))  # [co, oc, m]
    gpr_t = np.ascontiguousarray(
        (gprime / SOUT).reshape(4, 2, 128).transpose(2, 1, 0))
    w1t = np.ascontiguousarray(
        np.asarray(fc1_w, np.float32).T.reshape(2, 128, 32).transpose(1, 0, 2))
    b1 = np.asarray(fc1_b, np.float32).reshape(32, 1).copy()
    w2t = np.ascontiguousarray(
        np.asarray(fc2_w, np.float32).reshape(4, 2, 128, 32)
        .transpose(3, 0, 1, 2))
    b2t = np.ascontiguousarray(
        np.asarray(fc2_b, np.float32).reshape(4, 2, 128).transpose(2, 1, 0))

    shared = dict(w8h=w8h, w8l=w8l, bias_act=bias_t, gpr=gpr_t, w1t=w1t,
                  b1=b1, w2t=w2t, b2t=b2t)

    def slab4(arr, i0):
        s = arr[:, :, i0:i0 + 16, :]                 # [b, C, 16, 57]
        return np.ascontiguousarray(
            s.reshape(B, 2, 128, 16, 57).transpose(2, 0, 1, 3, 4)
            .reshape(128, 4, 16, 57))

    in_maps = []
    for k in range(NCORES):
        i0 = I0S[k]
        m = dict(shared)
        m["xs"] = slab4(x_ext, i0)
        m["x8h"] = slab4(x8h, i0)
        m["x8l"] = slab4(x8l, i0)
        m["m7"] = np.full((128, 1), 0.0 if k == NCORES - 1 else 1.0,
                          np.float32)
        in_maps.append(m)
    return in_maps


def kernel(x, conv_w, bn_gamma, bn_beta, bn_mean, bn_var, fc1_w, fc1_b,
           fc2_w, fc2_b):
    from concourse import bass_utils

    nc = _build_program()
    in_maps = _host_prep(x, conv_w, bn_gamma, bn_beta, bn_mean, bn_var,
                         fc1_w, fc1_b, fc2_w, fc2_b)
    res = bass_utils.run_bass_kernel_spmd(nc, in_maps,
                                          core_ids=list(range(NCORES)))
    full = np.zeros((B, C, HOUT, HOUT), np.float32)
    for k in range(NCORES):
        o = res.results[k]["out"]                     # [128, 4, 18, 142]
        o = o.reshape(128, B, 2, NROWS, HOUT).transpose(1, 2, 0, 3, 4) \
             .reshape(B, C, NROWS, HOUT) * np.float32(1.0 / SOUT)
        if k < 7:
            full[:, :, 18 * k:18 * k + 18, :] = o
        else:
            full[:, :, 126:142, :] = o[:, :, 0:16, :]
    return full
